# revision 19
# baseline (speedup 1.0000x reference)
"""Binarized-MLP (BNN) kernel for Trainium2, data-parallel over batch on 8 NeuronCores.

Reference computation:
    h      = x @ sign(W1) + b1          x:[8192,4096] W1:[4096,512]
    logits = sign(h) @ sign(W2) + b2    W2:[512,10]
    out    = softmax(logits)            [8192,10]

Per-core strategy (batch shard of 1024 rows):
  - The dominant matmul runs as an fp16 "hi" pass plus an fp8-e4m3 "lo"
    residual pass in DoubleRow perf mode (2 fp8 rows per PE cell, 256-row
    contraction per matmul). lo = (x - fp16(x)) * 256 is host-quantized to
    e4m3; the 1/256 un-scale is folded into the lo-pass stationary weights
    (sign(W1) * 2^-8, exact as an e4m3 subnormal — verified exact on HW).
    Both passes accumulate into the same fp32 PSUM banks. End-to-end rel
    err ~8e-3 (gate 2e-2) with bit-identical host quantization.
  - Weights are sign()ed on the host (free) — no device-side weight prep.
  - hi pass: stationary = sign(W1) f-tile [128f x 128j], moving = xT hi
    f-tile [128f x 512b] -> 8 PSUM banks hold h [512j x 1024b]; 256 MMs,
    f-major, fully dense on TensorE. Each stationary feeds 2 MMs (bc=0,1)
    — one LDWEIGHTS per 2 MMs is the most the PE pipeline can hide.
  - lo pass: stationary = 3D AP [128, 2, 128] fp8 pair-tile, moving =
    [128, 2, 512] fp8 -> 128 DoubleRow MMs, bank-major so each PSUM bank
    finishes early and sign/mm2/softmax overlap the remaining MMs.
  - sign(h)+b1 fused into one ScalarE Sign-activation (bias=b1) from PSUM
    into bf16 SBUF tiles [j, b], the stationary operand of the 2nd matmul.
  - softmax per b-tile: add b2, reduce_max(negate), Exp(bias=-max), DVE
    row-sum + reciprocal + scalar-mul (no Ln/second-Exp, no ACT
    accumulator read), output DMA per 2 b-tiles so ~10KB trails the end.
  - DMA count is kept low (25 total): weights ride in the same dram rows
    as x (one DMA per quad), the whole fp8 side is 2 DMAs, consts 2,
    outputs 4. Fewer DMAs = fewer 0.65us DIRECT2D issues on the sync ring
    and a shorter Tile exit drain.
  - Startup: first-quad DMAs are (i)-granular with weights on the scalar
    engine's HWDGE ring (parallel to sync's), and ~8 dummy matmuls keep
    the PE busy from ~7us so the HAM clock-gate hits 2.4GHz right as the
    first real data lands (~11.5us).
"""

import numpy as np
import ml_dtypes

import concourse.bass as bass
import concourse.tile as tile
from concourse import mybir
from concourse.bass_utils import run_bass_kernel_spmd
from bass_rust import ScopedClock, VectorClock

_CLEAR_SEMS = False
_EXIT_BARRIER = False

BF16 = mybir.dt.bfloat16
FP16 = mybir.dt.float16
F32 = mybir.dt.float32
FP8 = mybir.dt.float8e4

B, F, H, C = 8192, 4096, 512, 10
NCORES = 8
BC = B // NCORES          # 1024 batch rows per core
NF = F // 128             # 32 f-tiles (contraction)
NJ = H // 128             # 4 j-tiles (hidden)
NBC = BC // 512           # 2 moving-operand chunks of 512
NBT = BC // 128           # 8 output b-tiles
NQ = NF // 4              # 8 hi-pass quads (4 f-tiles per DMA)
NP = NF // 2              # 16 lo-pass pair-tiles (DoubleRow: 2 f-tiles/MM)

LO_SCALE = 256.0          # lo residual pre-scale (dodges e4m3 subnormals)
W_LO = 1.0 / LO_SCALE     # folded into lo-pass weights; 2^-8 exact in e4m3

QROW = 4 * H + 4 * BC     # merged quad row: [w: 4x512j][x: 4x1024b] fp16
PROW = H + BC             # merged fp8 pair-tile chunk: [w: 512j][x: 1024b]


class _PatchedTileContext(tile.TileContext):
    """Workaround for the walrus build in this container only accepting one
    sem wait on a CTRL-type (Drain) instruction: spread the exit drain's
    per-proc waits across several NOPs with one wait each. Also trims the
    exit: no closing all-engine barrier / sem clear (each engine halts after
    its own stream; the NOP waits already cover DMA completion lanes)."""

    def _drain_and_barrier(self, tick_clock, wait_clock):
        gc = tick_clock.global_clock
        ticks = list(gc)
        nprocs = len(ticks)
        engines = [
            self.nc.sync,
            self.nc.gpsimd,
            self.nc.vector,
            self.nc.scalar,
            self.nc.tensor,
        ]
        k = 0
        for i, t in enumerate(ticks):
            if t == 0:
                continue
            partial = [0] * nprocs
            partial[i] = t
            inst = engines[k % len(engines)].nop()
            k += 1
            wait_clock.add_sem_waits(
                inst.ins, ScopedClock({None: VectorClock(partial)})
            )
        self.nc.sync.drain()

        if _EXIT_BARRIER:
            self.nc.all_engine_barrier()
        assert self.sems is not None
        popped = self.nc._tile_sem_poison_stack.pop()
        assert popped is self._sem_poison
        if _CLEAR_SEMS:
            self.nc.clear_and_free_semaphores(list(self.sems.allocated().values()))


def _split_waits_json(raw: bytes) -> bytes:
    """The walrus build in this container accepts at most ONE sem wait per
    instruction (bass's own wait_op asserts the same). Tile attaches several.
    Rewrite the serialized BIR: excess waits become standalone EventSemaphore
    wait instructions on the same engine immediately before the instruction —
    semantically identical, since the engine blocks there first."""
    import json as _json

    m = _json.loads(raw)
    ctr = 0
    for fn in m.get("functions", []):
        for bb in fn.get("blocks", []):
            insts = bb.get("instructions", [])
            new_insts = []
            for inst in insts:
                si = inst.get("sync_info")
                waits = si.get("on_wait") or [] if si else []
                if len(waits) > 1:
                    for w in waits[:-1]:
                        new_insts.append(
                            {
                                "debug": inst.get("debug", 0),
                                "engine": inst["engine"],
                                "ins": [],
                                "outs": [],
                                "name": f"WSPLIT-{ctr}",
                                "opcode": "EventSemaphore",
                                "sync_info": {"on_update": [], "on_wait": [w]},
                            }
                        )
                        ctr += 1
                    si["on_wait"] = [waits[-1]]
                new_insts.append(inst)
            bb["instructions"] = new_insts
    return _json.dumps(m).encode()


def _install_wait_splitter(nc: bass.Bass) -> None:
    orig = nc.to_json_bytes

    def patched():
        return _split_waits_json(orig())

    nc.to_json_bytes = patched


def build_kernel() -> bass.Bass:
    nc = bass.Bass()
    # merged hi stream: row q*128+p = [w1 i0..3: 512j each][x i0..3: 1024b each]
    xwq = nc.dram_tensor("xwq", [NQ * 128, QROW], FP16, kind="ExternalInput")
    # merged fp8 side, per-partition [t=0..15][i=0..1][w1*2^-8: 512j | x lo: 1024b]
    xw8 = nc.dram_tensor("xw8", [128, NP * 2 * PROW], FP8, kind="ExternalInput")
    # f32 consts: [b1: 4][b2: 10]
    cst = nc.dram_tensor("cst", [128, NJ + C], F32, kind="ExternalInput")
    # sign(W2) bf16: w2s[p, j*C+c] = sign(W2)[j*128+p, c]
    w2sd = nc.dram_tensor("w2sd", [128, NJ * C], BF16, kind="ExternalInput")
    # packed per-core output [p, bt*10+c]; host reorders to [1024, 10]
    out = nc.dram_tensor("out", [128, NBT * C], F32, kind="ExternalOutput")

    with _PatchedTileContext(nc) as tc:
        with (
            tc.tile_pool(name="consts", bufs=1) as consts,
            tc.tile_pool(name="xwp", bufs=NQ) as xw_pool,
            tc.tile_pool(name="signh", bufs=NJ * NBC) as signh_pool,
            tc.tile_pool(name="psum", bufs=8, space="PSUM") as psum_pool,
            tc.tile_pool(name="smx", bufs=10) as smx_pool,
        ):
            # allocation order bc-major: pool ring slots 0-3 = bc0 banks,
            # 4-7 = bc1, so each bc's four psD re-allocations alias banks
            # already freed by that bc's own sign() reads.
            _ps = [
                psum_pool.tile([128, 512], F32, name="psB", tag="psB")
                for _ in range(NJ * NBC)
            ]
            psumB = [[_ps[bc * NJ + j] for bc in range(NBC)] for j in range(NJ)]

            # HAM warmup: dummy matmuls (into bank 0, overwritten by the
            # first real start=True matmul) keep the PE-busy window alive
            # from ~7us until the first real data lands (~11.5us), so the
            # clock-gate is at 2.4GHz when the real stream begins.
            warm = consts.tile([128, 640], FP16, name="warm", tag="warm")
            nc.gpsimd.memset(warm[:], 0.0)
            for _ in range(8):
                nc.tensor.matmul(
                    psumB[0][0][:], warm[:, :128], warm[:, 128:640],
                    start=True, stop=True,
                )

            def quad_in(q):
                xwt = xw_pool.tile([128, QROW], FP16, name="xwt", tag="xwt")
                row = xwq[q * 128:(q + 1) * 128, :]
                nc.sync.dma_start(xwt[:], row)
                return xwt

            # startup quads: i-granular in consumption order, with weights on
            # the scalar-engine HWDGE ring (issues in parallel with sync's)
            # and q1's transfers interleaved between q0's so each chunk lands
            # just ahead of the matmuls that consume it
            with tc.high_priority():
                xwt0 = xw_pool.tile([128, QROW], FP16, name="xwt", tag="xwt")
                xwt1 = xw_pool.tile([128, QROW], FP16, name="xwt", tag="xwt")
                row0 = xwq[0:128, :]
                row1 = xwq[128:256, :]
                # sync-ring issue order interleaves q1's x halves between
                # q0's so each chunk completes just ahead of its matmuls
                # (sync issues cost ~0.65us each; completion adds ~2-3us)
                for i in range(2):
                    nc.scalar.dma_start(
                        xwt0[:, i * 512:(i + 1) * 512],
                        row0[:, i * 512:(i + 1) * 512],
                    )
                    nc.sync.dma_start(
                        xwt0[:, 4 * H + i * BC:4 * H + (i + 1) * BC],
                        row0[:, 4 * H + i * BC:4 * H + (i + 1) * BC],
                    )
                nc.sync.dma_start(
                    xwt1[:, 4 * H:4 * H + 2 * BC], row1[:, 4 * H:4 * H + 2 * BC]
                )
                for i in range(2, 4):
                    nc.scalar.dma_start(
                        xwt0[:, i * 512:(i + 1) * 512],
                        row0[:, i * 512:(i + 1) * 512],
                    )
                    nc.sync.dma_start(
                        xwt0[:, 4 * H + i * BC:4 * H + (i + 1) * BC],
                        row0[:, 4 * H + i * BC:4 * H + (i + 1) * BC],
                    )
                nc.scalar.dma_start(xwt1[:, 0:4 * H], row1[:, 0:4 * H])
                nc.sync.dma_start(
                    xwt1[:, 4 * H + 2 * BC:QROW], row1[:, 4 * H + 2 * BC:QROW]
                )

            # ---- hi pass: fp16, f-major over all 8 banks ----
            b1b2 = w2s = None
            for q in range(NQ):
                if q == 0:
                    xwt = xwt0
                elif q == 1:
                    xwt = xwt1
                else:
                    xwt = quad_in(q)
                if q == 1:
                    b1b2 = consts.tile([128, NJ + C], F32, name="b1b2", tag="b1b2")
                    nc.sync.dma_start(b1b2[:], cst[:, :])
                    w2s = consts.tile([128, NJ * C], BF16, name="w2s", tag="w2s")
                    nc.sync.dma_start(w2s[:], w2sd[:, :])
                for i in range(4):
                    for j in range(NJ):
                        for bc in range(NBC):
                            nc.tensor.matmul(
                                psumB[j][bc][:],
                                xwt[:, i * 512 + j * 128:i * 512 + (j + 1) * 128],
                                xwt[:, 4 * H + i * BC + bc * 512:
                                    4 * H + i * BC + (bc + 1) * 512],
                                start=(q == 0 and i == 0),
                                stop=False,
                            )

            # fp8 side (lo-pass weights + x), 2 DMAs; consumed from ~60us
            xw8t = consts.tile([128, NP, 2, PROW], FP8, name="xw8t", tag="xw8t")
            xw8half = NP * PROW
            nc.sync.dma_start(xw8t[:, 0:NP // 2], xw8[:, 0:xw8half])
            nc.sync.dma_start(xw8t[:, NP // 2:NP], xw8[:, xw8half:2 * xw8half])

            # ---- lo pass: fp8 DoubleRow, bank-major; sign/mm2/softmax
            #      overlap the remaining DR MMs ----
            collect = smx_pool.tile([128, NBT * C], F32, name="collect", tag="collect")
            for bc in range(NBC):
                signh = [None] * NJ
                for j in range(NJ):
                    for t in range(NP):
                        nc.tensor.matmul(
                            psumB[j][bc][:],
                            xw8t[:, t, :, j * 128:(j + 1) * 128],
                            xw8t[:, t, :, H + bc * 512:H + (bc + 1) * 512],
                            start=False,
                            stop=(t == NP - 1),
                            perf_mode=mybir.MatmulPerfMode.DoubleRow,
                        )
                    s = signh_pool.tile([128, 512], BF16, name="signh", tag="signh")
                    nc.scalar.sign(s[:], psumB[j][bc][:], bias=b1b2[:, j:j + 1])
                    signh[j] = s
                for bt in range(4):
                    gbt = bc * 4 + bt
                    # one bank per b-tile (PSUM start=True zeroing is coarser
                    # than 40B, so logit groups can't share a bank); aliases a
                    # bank this bc's signs already freed
                    ps2 = psum_pool.tile([128, C], F32, name="psD", tag="psB")
                    for j in range(NJ):
                        nc.tensor.matmul(
                            ps2[:],
                            signh[j][:, bt * 128:(bt + 1) * 128],
                            w2s[:, j * C:(j + 1) * C],
                            start=(j == 0),
                            stop=(j == NJ - 1),
                        )
                    logits = smx_pool.tile([128, C], F32, name="logits", tag="logits")
                    nc.vector.tensor_add(logits[:], ps2[:], b1b2[:, NJ:NJ + C])
                    negmax = smx_pool.tile([128, 1], F32, name="negmax", tag="negmax")
                    nc.vector.reduce_max(
                        negmax[:], logits[:], axis=mybir.AxisListType.X, negate=True
                    )
                    e = smx_pool.tile([128, C], F32, name="e", tag="e")
                    nc.scalar.activation(
                        e[:],
                        logits[:],
                        mybir.ActivationFunctionType.Exp,
                        bias=negmax[:],
                    )
                    ssum = smx_pool.tile([128, 1], F32, name="ssum", tag="ssum")
                    nc.vector.tensor_reduce(
                        ssum[:], e[:], axis=mybir.AxisListType.X,
                        op=mybir.AluOpType.add,
                    )
                    rinv = smx_pool.tile([128, 1], F32, name="rinv", tag="rinv")
                    nc.vector.reciprocal(rinv[:], ssum[:])
                    nc.vector.tensor_scalar_mul(
                        collect[:, gbt * C:(gbt + 1) * C],
                        e[:],
                        rinv[:],
                    )
                    if bt % 2 == 1:
                        nc.sync.dma_start(
                            out[:, (gbt - 1) * C:(gbt + 1) * C],
                            collect[:, (gbt - 1) * C:(gbt + 1) * C],
                        )

    _install_wait_splitter(nc)
    return nc


_cached_nc = None


def _get_nc() -> bass.Bass:
    global _cached_nc
    if _cached_nc is None:
        _cached_nc = build_kernel()
    return _cached_nc


def kernel(inputs, W1, b1, W2, b2):
    x = np.ascontiguousarray(np.asarray(inputs, dtype=np.float32))
    W1 = np.asarray(W1, dtype=np.float32)
    b1 = np.asarray(b1, dtype=np.float32)
    W2 = np.asarray(W2, dtype=np.float32)
    b2 = np.asarray(b2, dtype=np.float32)

    w1s = np.where(W1 >= 0, np.float32(1.0), np.float32(-1.0))
    # [4096, 512] -> quad-packed [NQ, 128, 4*512] fp16
    w1h_pack = (
        w1s.astype(np.float16)
        .reshape(NQ, 4, 128, H).transpose(0, 2, 1, 3).reshape(NQ, 128, 4 * H)
    )
    # lo-pass weights: sign(W1)*2^-8, [NP, 2, 128, 512] fp8
    w1l_pack = (w1s * W_LO).astype(ml_dtypes.float8_e4m3).reshape(NP, 2, 128, H)
    b1b2 = np.ascontiguousarray(np.concatenate(
        [b1.reshape(NJ, 128).T, np.broadcast_to(b2.reshape(1, C), (128, C))],
        axis=1,
    ).astype(np.float32))
    w2s_pack = np.ascontiguousarray(
        np.where(W2 >= 0, np.float32(1.0), np.float32(-1.0))
        .astype(ml_dtypes.bfloat16)
        .reshape(NJ, 128, C).transpose(1, 0, 2).reshape(128, NJ * C)
    )

    in_maps = []
    for c in range(NCORES):
        xc_t = x[c * BC:(c + 1) * BC, :].T  # [F, BC] fp32
        hi = xc_t.astype(np.float16)
        lo8 = ((xc_t - hi.astype(np.float32)) * LO_SCALE).astype(
            ml_dtypes.float8_e4m3
        )
        # merged hi stream rows: [w1 quad | x quad]
        xh = hi.reshape(NQ, 4, 128, BC).transpose(0, 2, 1, 3).reshape(NQ, 128, 4 * BC)
        xwq_pack = np.concatenate([w1h_pack, xh], axis=2).reshape(NQ * 128, QROW)
        # merged fp8 rows per partition: [t][i][w1*2^-8 512 | x lo 1024]
        xl = lo8.reshape(NP, 2, 128, BC)
        xw8_pack = (
            np.concatenate([w1l_pack, xl], axis=3)  # [NP, 2, 128, PROW]
            .transpose(2, 0, 1, 3).reshape(128, NP * 2 * PROW)
        )
        in_maps.append(
            {
                "xwq": np.ascontiguousarray(xwq_pack),
                "xw8": np.ascontiguousarray(xw8_pack),
                "cst": b1b2,
                "w2sd": w2s_pack,
            }
        )

    nc = _get_nc()
    res = run_bass_kernel_spmd(nc, in_maps, core_ids=list(range(NCORES)))
    global last_results
    last_results = res
    parts = []
    for c in range(NCORES):
        oc = res.results[c]["out"]  # [128, NBT*C]
        parts.append(
            oc.reshape(128, NBT, C).transpose(1, 0, 2).reshape(BC, C)
        )
    return np.concatenate(parts, axis=0).astype(np.float32)


last_results = None


# revision 20
# speedup vs baseline: 1.0214x; 1.0214x over previous
"""Binarized-MLP (BNN) kernel for Trainium2, data-parallel over batch on 8 NeuronCores.

Reference computation:
    h      = x @ sign(W1) + b1          x:[8192,4096] W1:[4096,512]
    logits = sign(h) @ sign(W2) + b2    W2:[512,10]
    out    = softmax(logits)            [8192,10]

Per-core strategy (batch shard of 1024 rows):
  - The dominant matmul runs as an fp16 "hi" pass plus an fp8-e4m3 "lo"
    residual pass in DoubleRow perf mode (2 fp8 rows per PE cell, 256-row
    contraction per matmul). lo = (x - fp16(x)) * 256 is host-quantized to
    e4m3; the 1/256 un-scale is folded into the lo-pass stationary weights
    (sign(W1) * 2^-8, exact as an e4m3 subnormal — verified exact on HW).
    Both passes accumulate into the same fp32 PSUM banks. End-to-end rel
    err ~8e-3 (gate 2e-2) with bit-identical host quantization.
  - Weights are sign()ed on the host (free) — no device-side weight prep.
  - hi pass: stationary = sign(W1) f-tile [128f x 128j], moving = xT hi
    f-tile [128f x 512b] -> 8 PSUM banks hold h [512j x 1024b]; 256 MMs,
    f-major, fully dense on TensorE. Each stationary feeds 2 MMs (bc=0,1)
    — one LDWEIGHTS per 2 MMs is the most the PE pipeline can hide.
  - lo pass: stationary = 3D AP [128, 2, 128] fp8 pair-tile, moving =
    [128, 2, 512] fp8 -> 128 DoubleRow MMs, bank-major so each PSUM bank
    finishes early and sign/mm2/softmax overlap the remaining MMs.
  - sign(h)+b1 fused into one ScalarE Sign-activation (bias=b1) from PSUM
    into bf16 SBUF tiles [j, b], the stationary operand of the 2nd matmul.
  - softmax per b-tile: add b2, reduce_max(negate), Exp(bias=-max), DVE
    row-sum + reciprocal + scalar-mul (no Ln/second-Exp, no ACT
    accumulator read), output DMA per 2 b-tiles so ~10KB trails the end.
  - DMA count is kept low (25 total): weights ride in the same dram rows
    as x (one DMA per quad), the whole fp8 side is 2 DMAs, consts 2,
    outputs 4. Fewer DMAs = fewer 0.65us DIRECT2D issues on the sync ring
    and a shorter Tile exit drain.
  - Startup: first-quad DMAs are (i)-granular with weights on the scalar
    engine's HWDGE ring (parallel to sync's), and ~10 dummy matmuls keep
    the PE busy from ~7us so the HAM clock-gate hits 2.4GHz right as the
    first real data lands (~11.5us).
"""

import numpy as np
import ml_dtypes

import concourse.bass as bass
import concourse.tile as tile
from concourse import mybir
from concourse.bass_utils import run_bass_kernel_spmd
from bass_rust import ScopedClock, VectorClock

_CLEAR_SEMS = False
_EXIT_BARRIER = False

BF16 = mybir.dt.bfloat16
FP16 = mybir.dt.float16
F32 = mybir.dt.float32
FP8 = mybir.dt.float8e4

B, F, H, C = 8192, 4096, 512, 10
NCORES = 8
BC = B // NCORES          # 1024 batch rows per core
NF = F // 128             # 32 f-tiles (contraction)
NJ = H // 128             # 4 j-tiles (hidden)
NBC = BC // 512           # 2 moving-operand chunks of 512
NBT = BC // 128           # 8 output b-tiles
NQ = NF // 4              # 8 hi-pass quads (4 f-tiles per DMA)
NP = NF // 2              # 16 lo-pass pair-tiles (DoubleRow: 2 f-tiles/MM)

LO_SCALE = 256.0          # lo residual pre-scale (dodges e4m3 subnormals)
W_LO = 1.0 / LO_SCALE     # folded into lo-pass weights; 2^-8 exact in e4m3

QROW = 4 * H + 4 * BC     # merged quad row: [w: 4x512j][x: 4x1024b] fp16
PROW = H + BC             # merged fp8 pair-tile chunk: [w: 512j][x: 1024b]


class _PatchedTileContext(tile.TileContext):
    """Workaround for the walrus build in this container only accepting one
    sem wait on a CTRL-type (Drain) instruction: spread the exit drain's
    per-proc waits across several NOPs with one wait each. Also trims the
    exit: no closing all-engine barrier / sem clear (each engine halts after
    its own stream; the NOP waits already cover DMA completion lanes)."""

    def _drain_and_barrier(self, tick_clock, wait_clock):
        gc = tick_clock.global_clock
        ticks = list(gc)
        nprocs = len(ticks)
        engines = [
            self.nc.sync,
            self.nc.gpsimd,
            self.nc.vector,
            self.nc.scalar,
            self.nc.tensor,
        ]
        k = 0
        for i, t in enumerate(ticks):
            if t == 0:
                continue
            partial = [0] * nprocs
            partial[i] = t
            inst = engines[k % len(engines)].nop()
            k += 1
            wait_clock.add_sem_waits(
                inst.ins, ScopedClock({None: VectorClock(partial)})
            )
        self.nc.sync.drain()

        if _EXIT_BARRIER:
            self.nc.all_engine_barrier()
        assert self.sems is not None
        popped = self.nc._tile_sem_poison_stack.pop()
        assert popped is self._sem_poison
        if _CLEAR_SEMS:
            self.nc.clear_and_free_semaphores(list(self.sems.allocated().values()))


def _split_waits_json(raw: bytes) -> bytes:
    """The walrus build in this container accepts at most ONE sem wait per
    instruction (bass's own wait_op asserts the same). Tile attaches several.
    Rewrite the serialized BIR: excess waits become standalone EventSemaphore
    wait instructions on the same engine immediately before the instruction —
    semantically identical, since the engine blocks there first."""
    import json as _json

    m = _json.loads(raw)
    ctr = 0
    for fn in m.get("functions", []):
        for bb in fn.get("blocks", []):
            insts = bb.get("instructions", [])
            new_insts = []
            for inst in insts:
                si = inst.get("sync_info")
                waits = si.get("on_wait") or [] if si else []
                if len(waits) > 1:
                    for w in waits[:-1]:
                        new_insts.append(
                            {
                                "debug": inst.get("debug", 0),
                                "engine": inst["engine"],
                                "ins": [],
                                "outs": [],
                                "name": f"WSPLIT-{ctr}",
                                "opcode": "EventSemaphore",
                                "sync_info": {"on_update": [], "on_wait": [w]},
                            }
                        )
                        ctr += 1
                    si["on_wait"] = [waits[-1]]
                new_insts.append(inst)
            bb["instructions"] = new_insts
    return _json.dumps(m).encode()


def _install_wait_splitter(nc: bass.Bass) -> None:
    orig = nc.to_json_bytes

    def patched():
        return _split_waits_json(orig())

    nc.to_json_bytes = patched


def build_kernel() -> bass.Bass:
    nc = bass.Bass()
    # merged hi stream: row q*128+p = [w1 i0..3: 512j each][x i0..3: 1024b each]
    xwq = nc.dram_tensor("xwq", [NQ * 128, QROW], FP16, kind="ExternalInput")
    # merged fp8 side, per-partition [t=0..15][i=0..1][w1*2^-8: 512j | x lo: 1024b]
    xw8 = nc.dram_tensor("xw8", [128, NP * 2 * PROW], FP8, kind="ExternalInput")
    # f32 consts: [b1: 4][b2: 10]
    cst = nc.dram_tensor("cst", [128, NJ + C], F32, kind="ExternalInput")
    # sign(W2) bf16: w2s[p, j*C+c] = sign(W2)[j*128+p, c]
    w2sd = nc.dram_tensor("w2sd", [128, NJ * C], BF16, kind="ExternalInput")
    # packed per-core output [p, bt*10+c]; host reorders to [1024, 10]
    out = nc.dram_tensor("out", [128, NBT * C], F32, kind="ExternalOutput")

    with _PatchedTileContext(nc) as tc:
        with (
            tc.tile_pool(name="consts", bufs=1) as consts,
            tc.tile_pool(name="xwp", bufs=NQ) as xw_pool,
            tc.tile_pool(name="signh", bufs=NJ * NBC) as signh_pool,
            tc.tile_pool(name="psum", bufs=8, space="PSUM") as psum_pool,
            tc.tile_pool(name="smx", bufs=10) as smx_pool,
        ):
            # allocation order bc-major: pool ring slots 0-3 = bc0 banks,
            # 4-7 = bc1, so each bc's four psD re-allocations alias banks
            # already freed by that bc's own sign() reads.
            _ps = [
                psum_pool.tile([128, 512], F32, name="psB", tag="psB")
                for _ in range(NJ * NBC)
            ]
            psumB = [[_ps[bc * NJ + j] for bc in range(NBC)] for j in range(NJ)]

            # HAM warmup: dummy matmuls (into bank 0, overwritten by the
            # first real start=True matmul) keep the PE-busy window alive
            # from ~7us until the first real data lands (~11.5us), so the
            # clock-gate is at 2.4GHz when the real stream begins.
            warm = consts.tile([128, 640], FP16, name="warm", tag="warm")
            nc.gpsimd.memset(warm[:], 0.0)
            for _ in range(10):
                nc.tensor.matmul(
                    psumB[0][0][:], warm[:, :128], warm[:, 128:640],
                    start=True, stop=True,
                )

            def quad_in(q):
                xwt = xw_pool.tile([128, QROW], FP16, name="xwt", tag="xwt")
                row = xwq[q * 128:(q + 1) * 128, :]
                nc.sync.dma_start(xwt[:], row)
                return xwt

            # startup quads: i-granular in consumption order, with weights on
            # the scalar-engine HWDGE ring (issues in parallel with sync's)
            # and q1's transfers interleaved between q0's so each chunk lands
            # just ahead of the matmuls that consume it
            with tc.high_priority():
                xwt0 = xw_pool.tile([128, QROW], FP16, name="xwt", tag="xwt")
                xwt1 = xw_pool.tile([128, QROW], FP16, name="xwt", tag="xwt")
                row0 = xwq[0:128, :]
                row1 = xwq[128:256, :]
                # all of q0 first — any large transfer issued early steals
                # round-robin DMA bandwidth from the startup-critical chunks
                for i in range(4):
                    nc.scalar.dma_start(
                        xwt0[:, i * 512:(i + 1) * 512],
                        row0[:, i * 512:(i + 1) * 512],
                    )
                    nc.sync.dma_start(
                        xwt0[:, 4 * H + i * BC:4 * H + (i + 1) * BC],
                        row0[:, 4 * H + i * BC:4 * H + (i + 1) * BC],
                    )
                nc.scalar.dma_start(xwt1[:, 0:4 * H], row1[:, 0:4 * H])
                nc.sync.dma_start(
                    xwt1[:, 4 * H:4 * H + 2 * BC], row1[:, 4 * H:4 * H + 2 * BC]
                )
                nc.sync.dma_start(
                    xwt1[:, 4 * H + 2 * BC:QROW], row1[:, 4 * H + 2 * BC:QROW]
                )

            # ---- hi pass: fp16, f-major over all 8 banks ----
            b1b2 = w2s = None
            for q in range(NQ):
                if q == 0:
                    xwt = xwt0
                elif q == 1:
                    xwt = xwt1
                else:
                    xwt = quad_in(q)
                if q == 1:
                    b1b2 = consts.tile([128, NJ + C], F32, name="b1b2", tag="b1b2")
                    nc.sync.dma_start(b1b2[:], cst[:, :])
                    w2s = consts.tile([128, NJ * C], BF16, name="w2s", tag="w2s")
                    nc.sync.dma_start(w2s[:], w2sd[:, :])
                for i in range(4):
                    for j in range(NJ):
                        for bc in range(NBC):
                            nc.tensor.matmul(
                                psumB[j][bc][:],
                                xwt[:, i * 512 + j * 128:i * 512 + (j + 1) * 128],
                                xwt[:, 4 * H + i * BC + bc * 512:
                                    4 * H + i * BC + (bc + 1) * 512],
                                start=(q == 0 and i == 0),
                                stop=False,
                            )

            # fp8 side (lo-pass weights + x), 2 DMAs; consumed from ~60us
            xw8t = consts.tile([128, NP, 2, PROW], FP8, name="xw8t", tag="xw8t")
            xw8half = NP * PROW
            nc.sync.dma_start(xw8t[:, 0:NP // 2], xw8[:, 0:xw8half])
            nc.sync.dma_start(xw8t[:, NP // 2:NP], xw8[:, xw8half:2 * xw8half])

            # ---- lo pass: fp8 DoubleRow, bank-major; sign/mm2/softmax
            #      overlap the remaining DR MMs ----
            collect = smx_pool.tile([128, NBT * C], F32, name="collect", tag="collect")
            for bc in range(NBC):
                signh = [None] * NJ
                for j in range(NJ):
                    for t in range(NP):
                        nc.tensor.matmul(
                            psumB[j][bc][:],
                            xw8t[:, t, :, j * 128:(j + 1) * 128],
                            xw8t[:, t, :, H + bc * 512:H + (bc + 1) * 512],
                            start=False,
                            stop=(t == NP - 1),
                            perf_mode=mybir.MatmulPerfMode.DoubleRow,
                        )
                    s = signh_pool.tile([128, 512], BF16, name="signh", tag="signh")
                    nc.scalar.sign(s[:], psumB[j][bc][:], bias=b1b2[:, j:j + 1])
                    signh[j] = s
                for bt in range(4):
                    gbt = bc * 4 + bt
                    # one bank per b-tile (PSUM start=True zeroing is coarser
                    # than 40B, so logit groups can't share a bank); aliases a
                    # bank this bc's signs already freed
                    ps2 = psum_pool.tile([128, C], F32, name="psD", tag="psB")
                    for j in range(NJ):
                        nc.tensor.matmul(
                            ps2[:],
                            signh[j][:, bt * 128:(bt + 1) * 128],
                            w2s[:, j * C:(j + 1) * C],
                            start=(j == 0),
                            stop=(j == NJ - 1),
                        )
                    logits = smx_pool.tile([128, C], F32, name="logits", tag="logits")
                    nc.vector.tensor_add(logits[:], ps2[:], b1b2[:, NJ:NJ + C])
                    negmax = smx_pool.tile([128, 1], F32, name="negmax", tag="negmax")
                    nc.vector.reduce_max(
                        negmax[:], logits[:], axis=mybir.AxisListType.X, negate=True
                    )
                    e = smx_pool.tile([128, C], F32, name="e", tag="e")
                    nc.scalar.activation(
                        e[:],
                        logits[:],
                        mybir.ActivationFunctionType.Exp,
                        bias=negmax[:],
                    )
                    ssum = smx_pool.tile([128, 1], F32, name="ssum", tag="ssum")
                    nc.vector.tensor_reduce(
                        ssum[:], e[:], axis=mybir.AxisListType.X,
                        op=mybir.AluOpType.add,
                    )
                    rinv = smx_pool.tile([128, 1], F32, name="rinv", tag="rinv")
                    nc.vector.reciprocal(rinv[:], ssum[:])
                    nc.vector.tensor_scalar_mul(
                        collect[:, gbt * C:(gbt + 1) * C],
                        e[:],
                        rinv[:],
                    )
                    if bt % 2 == 1:
                        nc.sync.dma_start(
                            out[:, (gbt - 1) * C:(gbt + 1) * C],
                            collect[:, (gbt - 1) * C:(gbt + 1) * C],
                        )

    _install_wait_splitter(nc)
    return nc


_cached_nc = None


def _get_nc() -> bass.Bass:
    global _cached_nc
    if _cached_nc is None:
        _cached_nc = build_kernel()
    return _cached_nc


def kernel(inputs, W1, b1, W2, b2):
    x = np.ascontiguousarray(np.asarray(inputs, dtype=np.float32))
    W1 = np.asarray(W1, dtype=np.float32)
    b1 = np.asarray(b1, dtype=np.float32)
    W2 = np.asarray(W2, dtype=np.float32)
    b2 = np.asarray(b2, dtype=np.float32)

    w1s = np.where(W1 >= 0, np.float32(1.0), np.float32(-1.0))
    # [4096, 512] -> quad-packed [NQ, 128, 4*512] fp16
    w1h_pack = (
        w1s.astype(np.float16)
        .reshape(NQ, 4, 128, H).transpose(0, 2, 1, 3).reshape(NQ, 128, 4 * H)
    )
    # lo-pass weights: sign(W1)*2^-8, [NP, 2, 128, 512] fp8
    w1l_pack = (w1s * W_LO).astype(ml_dtypes.float8_e4m3).reshape(NP, 2, 128, H)
    b1b2 = np.ascontiguousarray(np.concatenate(
        [b1.reshape(NJ, 128).T, np.broadcast_to(b2.reshape(1, C), (128, C))],
        axis=1,
    ).astype(np.float32))
    w2s_pack = np.ascontiguousarray(
        np.where(W2 >= 0, np.float32(1.0), np.float32(-1.0))
        .astype(ml_dtypes.bfloat16)
        .reshape(NJ, 128, C).transpose(1, 0, 2).reshape(128, NJ * C)
    )

    in_maps = []
    for c in range(NCORES):
        xc_t = x[c * BC:(c + 1) * BC, :].T  # [F, BC] fp32
        hi = xc_t.astype(np.float16)
        lo8 = ((xc_t - hi.astype(np.float32)) * LO_SCALE).astype(
            ml_dtypes.float8_e4m3
        )
        # merged hi stream rows: [w1 quad | x quad]
        xh = hi.reshape(NQ, 4, 128, BC).transpose(0, 2, 1, 3).reshape(NQ, 128, 4 * BC)
        xwq_pack = np.concatenate([w1h_pack, xh], axis=2).reshape(NQ * 128, QROW)
        # merged fp8 rows per partition: [t][i][w1*2^-8 512 | x lo 1024]
        xl = lo8.reshape(NP, 2, 128, BC)
        xw8_pack = (
            np.concatenate([w1l_pack, xl], axis=3)  # [NP, 2, 128, PROW]
            .transpose(2, 0, 1, 3).reshape(128, NP * 2 * PROW)
        )
        in_maps.append(
            {
                "xwq": np.ascontiguousarray(xwq_pack),
                "xw8": np.ascontiguousarray(xw8_pack),
                "cst": b1b2,
                "w2sd": w2s_pack,
            }
        )

    nc = _get_nc()
    res = run_bass_kernel_spmd(nc, in_maps, core_ids=list(range(NCORES)))
    global last_results
    last_results = res
    parts = []
    for c in range(NCORES):
        oc = res.results[c]["out"]  # [128, NBT*C]
        parts.append(
            oc.reshape(128, NBT, C).transpose(1, 0, 2).reshape(BC, C)
        )
    return np.concatenate(parts, axis=0).astype(np.float32)


last_results = None


# revision 22
# speedup vs baseline: 1.0290x; 1.0074x over previous
"""Binarized-MLP (BNN) kernel for Trainium2, data-parallel over batch on 8 NeuronCores.

Reference computation:
    h      = x @ sign(W1) + b1          x:[8192,4096] W1:[4096,512]
    logits = sign(h) @ sign(W2) + b2    W2:[512,10]
    out    = softmax(logits)            [8192,10]

Per-core strategy (batch shard of 1024 rows):
  - The dominant matmul runs as an fp16 "hi" pass plus an fp8-e4m3 "lo"
    residual pass in DoubleRow perf mode (2 fp8 rows per PE cell, 256-row
    contraction per matmul). lo = (x - fp16(x)) * 256 is host-quantized to
    e4m3; the 1/256 un-scale is folded into the lo-pass stationary weights
    (sign(W1) * 2^-8, exact as an e4m3 subnormal — verified exact on HW).
    Both passes accumulate into the same fp32 PSUM banks. End-to-end rel
    err ~8e-3 (gate 2e-2) with bit-identical host quantization.
  - Weights are sign()ed on the host (free) — no device-side weight prep.
  - hi pass: stationary = sign(W1) f-tile [128f x 128j], moving = xT hi
    f-tile [128f x 512b] -> 8 PSUM banks hold h [512j x 1024b]; 256 MMs,
    f-major, fully dense on TensorE. Each stationary feeds 2 MMs (bc=0,1)
    — one LDWEIGHTS per 2 MMs is the most the PE pipeline can hide.
  - lo pass: stationary = 3D AP [128, 2, 128] fp8 pair-tile, moving =
    [128, 2, 512] fp8 -> 128 DoubleRow MMs, bank-major so each PSUM bank
    finishes early and sign/mm2/softmax overlap the remaining MMs.
  - sign(h)+b1 fused into one ScalarE Sign-activation (bias=b1) from PSUM
    into bf16 SBUF tiles [j, b], the stationary operand of the 2nd matmul.
  - softmax per b-tile: add b2, reduce_max(negate), Exp(bias=-max), DVE
    row-sum + reciprocal + scalar-mul (no Ln/second-Exp, no ACT
    accumulator read), output DMA per 2 b-tiles so ~10KB trails the end.
  - DMA count is kept low (25 total): weights ride in the same dram rows
    as x (one DMA per quad), the whole fp8 side is 2 DMAs, consts 2,
    outputs 4. Fewer DMAs = fewer 0.65us DIRECT2D issues on the sync ring
    and a shorter Tile exit drain.
  - Startup: first-quad DMAs are (i)-granular with weights on the scalar
    engine's HWDGE ring (parallel to sync's), and ~10 dummy matmuls keep
    the PE busy from ~7us so the HAM clock-gate hits 2.4GHz right as the
    first real data lands (~11.5us).
"""

import numpy as np
import ml_dtypes

import concourse.bass as bass
import concourse.tile as tile
from concourse import mybir
from concourse.bass_utils import run_bass_kernel_spmd
from bass_rust import ScopedClock, VectorClock

_CLEAR_SEMS = False
_EXIT_BARRIER = False

BF16 = mybir.dt.bfloat16
FP16 = mybir.dt.float16
F32 = mybir.dt.float32
FP8 = mybir.dt.float8e4

B, F, H, C = 8192, 4096, 512, 10
NCORES = 8
BC = B // NCORES          # 1024 batch rows per core
NF = F // 128             # 32 f-tiles (contraction)
NJ = H // 128             # 4 j-tiles (hidden)
NBC = BC // 512           # 2 moving-operand chunks of 512
NBT = BC // 128           # 8 output b-tiles
NQ = NF // 4              # 8 hi-pass quads (4 f-tiles per DMA)
NP = NF // 2              # 16 lo-pass pair-tiles (DoubleRow: 2 f-tiles/MM)

LO_SCALE = 256.0          # lo residual pre-scale (dodges e4m3 subnormals)
W_LO = 1.0 / LO_SCALE     # folded into lo-pass weights; 2^-8 exact in e4m3

QROW = 4 * H + 4 * BC     # merged quad row: [w: 4x512j][x: 4x1024b] fp16
PROW = H + BC             # merged fp8 pair-tile chunk: [w: 512j][x: 1024b]


class _PatchedTileContext(tile.TileContext):
    """Workaround for the walrus build in this container only accepting one
    sem wait on a CTRL-type (Drain) instruction: spread the exit drain's
    per-proc waits across several NOPs with one wait each. Also trims the
    exit: no closing all-engine barrier / sem clear (each engine halts after
    its own stream; the NOP waits already cover DMA completion lanes)."""

    def _drain_and_barrier(self, tick_clock, wait_clock):
        gc = tick_clock.global_clock
        ticks = list(gc)
        nprocs = len(ticks)
        engines = [
            self.nc.sync,
            self.nc.gpsimd,
            self.nc.vector,
            self.nc.scalar,
            self.nc.tensor,
        ]
        k = 0
        for i, t in enumerate(ticks):
            if t == 0:
                continue
            partial = [0] * nprocs
            partial[i] = t
            inst = engines[k % len(engines)].nop()
            k += 1
            wait_clock.add_sem_waits(
                inst.ins, ScopedClock({None: VectorClock(partial)})
            )
        self.nc.sync.drain()

        if _EXIT_BARRIER:
            self.nc.all_engine_barrier()
        assert self.sems is not None
        popped = self.nc._tile_sem_poison_stack.pop()
        assert popped is self._sem_poison
        if _CLEAR_SEMS:
            self.nc.clear_and_free_semaphores(list(self.sems.allocated().values()))


def _split_waits_json(raw: bytes) -> bytes:
    """The walrus build in this container accepts at most ONE sem wait per
    instruction (bass's own wait_op asserts the same). Tile attaches several.
    Rewrite the serialized BIR: excess waits become standalone EventSemaphore
    wait instructions on the same engine immediately before the instruction —
    semantically identical, since the engine blocks there first."""
    import json as _json

    m = _json.loads(raw)
    ctr = 0
    for fn in m.get("functions", []):
        for bb in fn.get("blocks", []):
            insts = bb.get("instructions", [])
            new_insts = []
            for inst in insts:
                si = inst.get("sync_info")
                waits = si.get("on_wait") or [] if si else []
                if len(waits) > 1:
                    for w in waits[:-1]:
                        new_insts.append(
                            {
                                "debug": inst.get("debug", 0),
                                "engine": inst["engine"],
                                "ins": [],
                                "outs": [],
                                "name": f"WSPLIT-{ctr}",
                                "opcode": "EventSemaphore",
                                "sync_info": {"on_update": [], "on_wait": [w]},
                            }
                        )
                        ctr += 1
                    si["on_wait"] = [waits[-1]]
                new_insts.append(inst)
            bb["instructions"] = new_insts
    return _json.dumps(m).encode()


def _install_wait_splitter(nc: bass.Bass) -> None:
    orig = nc.to_json_bytes

    def patched():
        return _split_waits_json(orig())

    nc.to_json_bytes = patched


def build_kernel() -> bass.Bass:
    nc = bass.Bass()
    # merged hi stream: row q*128+p = [w1 i0..3: 512j each][x i0..3: 1024b each]
    xwq = nc.dram_tensor("xwq", [NQ * 128, QROW], FP16, kind="ExternalInput")
    # merged fp8 side, per-partition [t=0..15][i=0..1][w1*2^-8: 512j | x lo: 1024b]
    xw8 = nc.dram_tensor("xw8", [128, NP * 2 * PROW], FP8, kind="ExternalInput")
    # f32 consts: [b1: 4][b2: 10]
    cst = nc.dram_tensor("cst", [128, NJ + C], F32, kind="ExternalInput")
    # sign(W2) bf16: w2s[p, j*C+c] = sign(W2)[j*128+p, c]
    w2sd = nc.dram_tensor("w2sd", [128, NJ * C], BF16, kind="ExternalInput")
    # packed per-core output [p, bt*10+c]; host reorders to [1024, 10]
    out = nc.dram_tensor("out", [128, NBT * C], F32, kind="ExternalOutput")

    with _PatchedTileContext(nc) as tc:
        with (
            tc.tile_pool(name="consts", bufs=1) as consts,
            tc.tile_pool(name="xwp", bufs=NQ) as xw_pool,
            tc.tile_pool(name="signh", bufs=NJ * NBC) as signh_pool,
            tc.tile_pool(name="psum", bufs=8, space="PSUM") as psum_pool,
            tc.tile_pool(name="smx", bufs=10) as smx_pool,
        ):
            # allocation order bc-major: pool ring slots 0-3 = bc0 banks,
            # 4-7 = bc1, so each bc's four psD re-allocations alias banks
            # already freed by that bc's own sign() reads.
            _ps = [
                psum_pool.tile([128, 512], F32, name="psB", tag="psB")
                for _ in range(NJ * NBC)
            ]
            psumB = [[_ps[bc * NJ + j] for bc in range(NBC)] for j in range(NJ)]

            # HAM warmup: dummy matmuls (into bank 0, overwritten by the
            # first real start=True matmul) keep the PE-busy window alive
            # from ~7us until the first real data lands (~11.5us), so the
            # clock-gate is at 2.4GHz when the real stream begins.
            warm = consts.tile([128, 640], FP16, name="warm", tag="warm")
            nc.gpsimd.memset(warm[:], 0.0)
            for _ in range(10):
                nc.tensor.matmul(
                    psumB[0][0][:], warm[:, :128], warm[:, 128:640],
                    start=True, stop=True,
                )

            def quad_in(q):
                xwt = xw_pool.tile([128, QROW], FP16, name="xwt", tag="xwt")
                row = xwq[q * 128:(q + 1) * 128, :]
                nc.sync.dma_start(xwt[:], row)
                return xwt

            # startup quads: i-granular in consumption order, with weights on
            # the scalar-engine HWDGE ring (issues in parallel with sync's)
            # and q1's transfers interleaved between q0's so each chunk lands
            # just ahead of the matmuls that consume it
            with tc.high_priority():
                xwt0 = xw_pool.tile([128, QROW], FP16, name="xwt", tag="xwt")
                xwt1 = xw_pool.tile([128, QROW], FP16, name="xwt", tag="xwt")
                row0 = xwq[0:128, :]
                row1 = xwq[128:256, :]
                # all of q0 first — any large transfer issued early steals
                # round-robin DMA bandwidth from the startup-critical chunks
                for i in range(4):
                    nc.scalar.dma_start(
                        xwt0[:, i * 512:(i + 1) * 512],
                        row0[:, i * 512:(i + 1) * 512],
                    )
                    nc.sync.dma_start(
                        xwt0[:, 4 * H + i * BC:4 * H + (i + 1) * BC],
                        row0[:, 4 * H + i * BC:4 * H + (i + 1) * BC],
                    )
                nc.scalar.dma_start(xwt1[:, 0:4 * H], row1[:, 0:4 * H])
                nc.sync.dma_start(
                    xwt1[:, 4 * H:4 * H + 2 * BC], row1[:, 4 * H:4 * H + 2 * BC]
                )
                nc.sync.dma_start(
                    xwt1[:, 4 * H + 2 * BC:QROW], row1[:, 4 * H + 2 * BC:QROW]
                )

            # ---- hi pass: fp16, f-major over all 8 banks ----
            b1b2 = w2s = None
            for q in range(NQ):
                if q == 0:
                    xwt = xwt0
                elif q == 1:
                    xwt = xwt1
                else:
                    xwt = quad_in(q)
                if q == 1:
                    b1b2 = consts.tile([128, NJ + C], F32, name="b1b2", tag="b1b2")
                    nc.sync.dma_start(b1b2[:], cst[:, :])
                    w2s = consts.tile([128, NJ * C], BF16, name="w2s", tag="w2s")
                    nc.sync.dma_start(w2s[:], w2sd[:, :])
                for i in range(4):
                    for j in range(NJ):
                        for bc in range(NBC):
                            nc.tensor.matmul(
                                psumB[j][bc][:],
                                xwt[:, i * 512 + j * 128:i * 512 + (j + 1) * 128],
                                xwt[:, 4 * H + i * BC + bc * 512:
                                    4 * H + i * BC + (bc + 1) * 512],
                                start=(q == 0 and i == 0),
                                stop=False,
                            )

            # fp8 side (lo-pass weights + x), 2 DMAs; consumed from ~60us
            xw8t = consts.tile([128, NP, 2, PROW], FP8, name="xw8t", tag="xw8t")
            xw8half = NP * PROW
            nc.sync.dma_start(xw8t[:, 0:NP // 2], xw8[:, 0:xw8half])
            nc.sync.dma_start(xw8t[:, NP // 2:NP], xw8[:, xw8half:2 * xw8half])

            # ---- lo pass: fp8 DoubleRow, bank-major; sign/mm2/softmax
            #      overlap the remaining DR MMs ----
            collect = smx_pool.tile([128, NBT * C], F32, name="collect", tag="collect")
            for bc in range(NBC):
                signh = [None] * NJ
                ps2s = [None] * 4
                done_j = [0] * 4

                def mm2_catchup(upto_j):
                    # run every second-layer mm whose signh already exists.
                    # Placed one j-block behind the sign that enables it, so
                    # the PE reaches these ~3.5us after the ScalarE sign was
                    # issued and never stalls the FIFO on it.
                    for bt in range(upto_j + 1):
                        if ps2s[bt] is None:
                            continue
                        for jj in range(done_j[bt], upto_j + 1):
                            nc.tensor.matmul(
                                ps2s[bt][:],
                                signh[jj][:, bt * 128:(bt + 1) * 128],
                                w2s[:, jj * C:(jj + 1) * C],
                                start=(jj == 0),
                                stop=(jj == NJ - 1),
                            )
                        done_j[bt] = upto_j + 1

                for j in range(NJ):
                    for t in range(NP):
                        nc.tensor.matmul(
                            psumB[j][bc][:],
                            xw8t[:, t, :, j * 128:(j + 1) * 128],
                            xw8t[:, t, :, H + bc * 512:H + (bc + 1) * 512],
                            start=False,
                            stop=(t == NP - 1),
                            perf_mode=mybir.MatmulPerfMode.DoubleRow,
                        )
                    if j >= 1:
                        mm2_catchup(j - 1)
                    s = signh_pool.tile([128, 512], BF16, name="signh", tag="signh")
                    nc.scalar.sign(s[:], psumB[j][bc][:], bias=b1b2[:, j:j + 1])
                    signh[j] = s
                    # one bank per b-tile (PSUM start=True zeroing is coarser
                    # than 40B, so logit groups can't share a bank); each
                    # sign frees exactly one bank, claimed for b-tile j here
                    ps2s[j] = psum_pool.tile([128, C], F32, name="psD", tag="psB")
                mm2_catchup(NJ - 1)
                for bt in range(4):
                    gbt = bc * 4 + bt
                    ps2 = ps2s[bt]
                    logits = smx_pool.tile([128, C], F32, name="logits", tag="logits")
                    nc.vector.tensor_add(logits[:], ps2[:], b1b2[:, NJ:NJ + C])
                    negmax = smx_pool.tile([128, 1], F32, name="negmax", tag="negmax")
                    nc.vector.reduce_max(
                        negmax[:], logits[:], axis=mybir.AxisListType.X, negate=True
                    )
                    e = smx_pool.tile([128, C], F32, name="e", tag="e")
                    nc.scalar.activation(
                        e[:],
                        logits[:],
                        mybir.ActivationFunctionType.Exp,
                        bias=negmax[:],
                    )
                    ssum = smx_pool.tile([128, 1], F32, name="ssum", tag="ssum")
                    nc.vector.tensor_reduce(
                        ssum[:], e[:], axis=mybir.AxisListType.X,
                        op=mybir.AluOpType.add,
                    )
                    rinv = smx_pool.tile([128, 1], F32, name="rinv", tag="rinv")
                    nc.vector.reciprocal(rinv[:], ssum[:])
                    nc.vector.tensor_scalar_mul(
                        collect[:, gbt * C:(gbt + 1) * C],
                        e[:],
                        rinv[:],
                    )
                    if bt % 2 == 1:
                        nc.sync.dma_start(
                            out[:, (gbt - 1) * C:(gbt + 1) * C],
                            collect[:, (gbt - 1) * C:(gbt + 1) * C],
                        )

    _install_wait_splitter(nc)
    return nc


_cached_nc = None


def _get_nc() -> bass.Bass:
    global _cached_nc
    if _cached_nc is None:
        _cached_nc = build_kernel()
    return _cached_nc


def kernel(inputs, W1, b1, W2, b2):
    x = np.ascontiguousarray(np.asarray(inputs, dtype=np.float32))
    W1 = np.asarray(W1, dtype=np.float32)
    b1 = np.asarray(b1, dtype=np.float32)
    W2 = np.asarray(W2, dtype=np.float32)
    b2 = np.asarray(b2, dtype=np.float32)

    w1s = np.where(W1 >= 0, np.float32(1.0), np.float32(-1.0))
    # [4096, 512] -> quad-packed [NQ, 128, 4*512] fp16
    w1h_pack = (
        w1s.astype(np.float16)
        .reshape(NQ, 4, 128, H).transpose(0, 2, 1, 3).reshape(NQ, 128, 4 * H)
    )
    # lo-pass weights: sign(W1)*2^-8, [NP, 2, 128, 512] fp8
    w1l_pack = (w1s * W_LO).astype(ml_dtypes.float8_e4m3).reshape(NP, 2, 128, H)
    b1b2 = np.ascontiguousarray(np.concatenate(
        [b1.reshape(NJ, 128).T, np.broadcast_to(b2.reshape(1, C), (128, C))],
        axis=1,
    ).astype(np.float32))
    w2s_pack = np.ascontiguousarray(
        np.where(W2 >= 0, np.float32(1.0), np.float32(-1.0))
        .astype(ml_dtypes.bfloat16)
        .reshape(NJ, 128, C).transpose(1, 0, 2).reshape(128, NJ * C)
    )

    in_maps = []
    for c in range(NCORES):
        xc_t = x[c * BC:(c + 1) * BC, :].T  # [F, BC] fp32
        hi = xc_t.astype(np.float16)
        lo8 = ((xc_t - hi.astype(np.float32)) * LO_SCALE).astype(
            ml_dtypes.float8_e4m3
        )
        # merged hi stream rows: [w1 quad | x quad]
        xh = hi.reshape(NQ, 4, 128, BC).transpose(0, 2, 1, 3).reshape(NQ, 128, 4 * BC)
        xwq_pack = np.concatenate([w1h_pack, xh], axis=2).reshape(NQ * 128, QROW)
        # merged fp8 rows per partition: [t][i][w1*2^-8 512 | x lo 1024]
        xl = lo8.reshape(NP, 2, 128, BC)
        xw8_pack = (
            np.concatenate([w1l_pack, xl], axis=3)  # [NP, 2, 128, PROW]
            .transpose(2, 0, 1, 3).reshape(128, NP * 2 * PROW)
        )
        in_maps.append(
            {
                "xwq": np.ascontiguousarray(xwq_pack),
                "xw8": np.ascontiguousarray(xw8_pack),
                "cst": b1b2,
                "w2sd": w2s_pack,
            }
        )

    nc = _get_nc()
    res = run_bass_kernel_spmd(nc, in_maps, core_ids=list(range(NCORES)))
    global last_results
    last_results = res
    parts = []
    for c in range(NCORES):
        oc = res.results[c]["out"]  # [128, NBT*C]
        parts.append(
            oc.reshape(128, NBT, C).transpose(1, 0, 2).reshape(BC, C)
        )
    return np.concatenate(parts, axis=0).astype(np.float32)


last_results = None
